# revision 17
# baseline (speedup 1.0000x reference)
"""ChunkFlowClassifier Trainium2 kernel — fp8 stream version.

Math (per sample, reference.py):
  L = sum(attention_mask); mid = L // 2
  first_pool  = mean(hidden[1:mid])        # [H]
  second_pool = mean(hidden[mid:L-1])      # [H]
  fh, sh = LN(first_pool), LN(second_pool)
  flow = [fh, sh, sh - fh]                 # [3H]
  out = gelu(gelu(flow @ W1 + b1) @ W2 + b2) @ W3 + b3   # [5]

Data-parallel over 8 NeuronCores (8 samples/core). Host packs only the
rows each sample actually uses (positions 1..L-2) into a dense fp8
(e4m3) buffer plus 0/1 mask columns routing each row into one of 16
(sample, half) PSUM accumulators; the device pools via DoubleRow fp8 PE
matmuls (2 k-tiles / instruction), then runs LayerNorm + MLP on-chip.

fp8 quantization uses sigma-delta error feedback along 64-row chains
(carry reset at group starts): within a chain the quantization errors
telescope, so each group's pooled SUM error is ~1 quantization ulp per
chain touching it instead of sqrt(N) ulps — pooled sums stay accurate
to ~0.2% even at 1 byte/element.

Host-side algebraic folds (exact, just reassociation):
  flow @ W1 = fh@(W1a - W1c) + sh@(W1b + W1c)        (W1 = [W1a; W1b; W1c])
  LN scale/shift:  (xhat*g + b) @ M = xhat @ (g[:,None]*M) + b @ M
  LN on raw sums (scale-invariant), eps scaled by count^2.
"""

import numpy as np
import ml_dtypes

B, S, H = 64, 2048, 768
NCORES = 8
SPC = 8              # samples per core
CHUNK = 16           # max 128-row tiles per DMA transfer (chunk = CHUNK*98KB)
XBUFS = 6            # SBUF buffering depth for stream chunks
RINGS = ("sync", "scalar")   # DMA queues the stream rotates over

F8 = ml_dtypes.float8_e4m3   # matches TRN FP8_EXP4 (dt.float8e4)

_NC_CACHE = {}

# fp16-bit -> e4m3 LUTs (built once): byte pattern + exact dequant value
_U16 = np.arange(65536, dtype=np.uint16)
with np.errstate(invalid="ignore", over="ignore"):
    _F8V = _U16.view(np.float16).astype(np.float32).astype(F8)
BYTE_LUT = _F8V.view(np.uint8)
DEQ_LUT = _F8V.astype(np.float32)
_ONE_BYTE = int(np.asarray(1.0, F8).view(np.uint8))


def _sched(NT):
    """Chunk schedule: list of (tile_start, ntiles), ntiles even."""
    out, t = [], 0
    while NT - t >= CHUNK:
        out.append((t, CHUNK))
        t += CHUNK
    if NT - t:
        out.append((t, NT - t))
    return out


def _build_nc(NT, repeat=1, unroll=1):
    import concourse.bacc as bacc
    import concourse.tile as tile
    from concourse import mybir

    dt = mybir.dt
    f32 = dt.float32
    f8 = dt.float8e4
    f16 = dt.float16
    Alu = mybir.AluOpType
    Act = mybir.ActivationFunctionType
    DR = mybir.MatmulPerfMode.DoubleRow

    sched = _sched(NT)
    npairs = NT // 2

    nc = bacc.Bacc("TRN2", target_bir_lowering=False, debug=False,
                   num_devices=NCORES)

    def din(name, shape, d=f32):
        return nc.dram_tensor(name, shape, d, kind="ExternalInput").ap()

    xin = din("xin", [128, NT * H], f8)
    mc = din("mc", [128, NT * 16], f8)
    epsc = din("epsc", [16, 1])
    idn = din("idn", [16, 16])
    w1 = din("w1", [2 * H, 512], f16)       # folded (see module docstring)
    b1 = din("b1", [1, 512], f16)
    w2 = din("w2", [512, 128], f16)
    b2 = din("b2", [1, 128])
    w3 = din("w3", [128, 5])
    b3 = din("b3", [1, 5])
    out = nc.dram_tensor("out", [SPC, 5], f32, kind="ExternalOutput").ap()

    with tile.TileContext(nc) as tc:
        with (
            tc.tile_pool(name="xp", bufs=XBUFS) as xp,
            tc.tile_pool(name="sg", bufs=1) as sg,
            tc.tile_pool(name="sm", bufs=1) as sm,
            tc.tile_pool(name="tpp", bufs=1, space="PSUM") as tpp,
            tc.tile_pool(name="mlp", bufs=1, space="PSUM") as mlp,
            tc.tile_pool(name="acc", bufs=1, space="PSUM") as acc,
        ):
            mc_sb = sg.tile([128, npairs, 2, 16], f8)
            nc.sync.dma_start(out=mc_sb,
                              in_=mc.rearrange("p (a b m) -> p a b m", b=2, m=16))
            # weights/constants: prefetch alongside the stream
            w1_sb = sg.tile([128, 12, 512], f16)
            nc.scalar.dma_start(out=w1_sb, in_=w1.rearrange("(k p) n -> p k n", p=128))
            w2_sb = sg.tile([128, 4, 128], f16)
            nc.scalar.dma_start(out=w2_sb, in_=w2.rearrange("(k p) n -> p k n", p=128))
            w3_sb = sg.tile([128, 5], f32)
            nc.scalar.dma_start(out=w3_sb, in_=w3)
            b1_sb = sm.tile([1, 512], f16)
            nc.scalar.dma_start(out=b1_sb, in_=b1)
            b2_sb = sm.tile([1, 128], f32)
            nc.scalar.dma_start(out=b2_sb, in_=b2)
            b3_sb = sm.tile([1, 5], f32)
            nc.scalar.dma_start(out=b3_sb, in_=b3)
            epsc_sb = sm.tile([16, 1], f32)
            nc.sync.dma_start(out=epsc_sb, in_=epsc)
            idn_sb = sm.tile([16, 16], f32)
            nc.sync.dma_start(out=idn_sb, in_=idn)
            idn16_sb = sm.tile([16, 16], f16)
            nc.vector.tensor_copy(idn16_sb, idn_sb)
            ones_sb = sm.tile([1, SPC], f16)
            nc.vector.memset(ones_sb, 1.0)
            onesf_sb = sm.tile([1, SPC], f32)
            nc.vector.memset(onesf_sb, 1.0)
            scr_sb = sm.tile([1, 2], f32)
            nc.vector.memset(scr_sb, 1.0)
            # touch Sqrt+Gelu once early so ACT table loads overlap the stream
            nc.scalar.activation(out=scr_sb[:, 0:1], in_=scr_sb[:, 0:1],
                                 func=Act.Sqrt)
            nc.scalar.activation(out=scr_sb[:, 1:2], in_=scr_sb[:, 1:2],
                                 func=Act.Gelu)

            # double-buffered accumulators: pass parity alternates bank sets so
            # the next pass's matmuls never WAR-stall on the LN chain's reads
            ps1s = [acc.tile([16, 512], f32, name=f"ps1_{i}") for i in range(2)]
            ps2s = [acc.tile([16, 256], f32, name=f"ps2_{i}") for i in range(2)]

            def stream_body(ps1, ps2):
                for gi, (t0, ct) in enumerate(sched):
                    xt = xp.tile([128, CHUNK, H], f8, tag="x")
                    eng = getattr(nc, RINGS[gi % len(RINGS)])
                    eng.dma_start(
                        out=xt[:, 0:ct, :],
                        in_=xin[:, t0 * H:(t0 + ct) * H]
                            .rearrange("p (c h) -> p c h", h=H))
                    for d in range(ct // 2):
                        t2 = t0 // 2 + d
                        first = t2 == 0
                        last = t2 == npairs - 1
                        lhs = mc_sb[:, t2]                       # [128, 2, 16]
                        xpair = xt[:, 2 * d:2 * d + 2, :]        # [128, 2, H]
                        nc.tensor.matmul(ps1, lhs, xpair[:, :, 0:512],
                                         start=first, stop=last, perf_mode=DR)
                        nc.tensor.matmul(ps2, lhs, xpair[:, :, 512:H],
                                         start=first, stop=last, perf_mode=DR)

            def full_pass(parity=0):
                ps1, ps2 = ps1s[parity], ps2s[parity]
                stream_body(ps1, ps2)
                # LayerNorm directly on the raw sums: LN is scale-invariant,
                # with eps scaled by cnt^2 (host-provided) to stay exact.
                stats = sm.tile([16, 3, 6], f32)
                nc.vector.bn_stats(out=stats[:, 0, :], in_=ps1[:, 0:256])
                nc.vector.bn_stats(out=stats[:, 1, :], in_=ps1[:, 256:512])
                nc.vector.bn_stats(out=stats[:, 2, :], in_=ps2)
                mv = sm.tile([16, 2], f32)
                nc.vector.bn_aggr(out=mv, in_=stats)
                rstd = sm.tile([16, 1], f32)
                nc.scalar.activation(out=rstd, in_=mv[:, 1:2], func=Act.Sqrt,
                                     bias=epsc_sb, scale=1.0)
                nc.vector.reciprocal(out=rstd, in_=rstd)
                xn1 = sg.tile([16, 512], f16)
                xn2 = sg.tile([16, 256], f16)
                nc.vector.tensor_scalar(out=xn1, in0=ps1, scalar1=mv[:, 0:1],
                                        scalar2=rstd, op0=Alu.subtract, op1=Alu.mult)
                nc.vector.tensor_scalar(out=xn2, in0=ps2, scalar1=mv[:, 0:1],
                                        scalar2=rstd, op0=Alu.subtract, op1=Alu.mult)

                # transpose the 16 normalized vectors -> 12 k-tiles [128, 8] fp16
                flowT = sg.tile([128, 12, SPC], f16)
                tp6 = tpp.tile([128, 6, 16], f32, tag="tp")
                for c6 in range(6):
                    src_ap = (xn1[:, c6 * 128:(c6 + 1) * 128] if c6 < 4
                              else xn2[:, (c6 - 4) * 128:(c6 - 3) * 128])
                    nc.tensor.matmul(tp6[:, c6, :], src_ap,
                                     idn16_sb, start=True, stop=True)
                # tp6[:, c, h*8:h*8+8] holds (half h, chunk c); flowT k-tile
                # order is [fh chunks 0..5 | sh chunks 0..5]
                nc.vector.tensor_copy(flowT[:, 0:6, :], tp6[:, :, 0:SPC])
                nc.vector.tensor_copy(flowT[:, 6:12, :], tp6[:, :, SPC:16])

                # layer 1: h1[8, 512] = gelu(fh @ W1f[:H] + sh @ W1f[H:] + b1f)
                h1ps = mlp.tile([SPC, 512], f32, tag="h1")
                for k in range(12):
                    nc.tensor.matmul(h1ps, flowT[:, k, :], w1_sb[:, k, :],
                                     start=(k == 0), stop=False)
                nc.tensor.matmul(h1ps, ones_sb, b1_sb, start=False, stop=True)
                h1 = sg.tile([SPC, 512], f16)
                nc.scalar.activation(out=h1, in_=h1ps, func=Act.Gelu)

                h1T = sg.tile([128, 4, SPC], f16)
                tp4 = tpp.tile([128, 4, SPC], f32, tag="tp")
                for k in range(4):
                    nc.tensor.matmul(tp4[:, k, :], h1[:, k * 128:(k + 1) * 128],
                                     idn16_sb[0:SPC, 0:SPC], start=True, stop=True)
                nc.vector.tensor_copy(h1T, tp4)

                # layer 2: h2[8, 128] = gelu(h1 @ W2 + b2)
                h2ps = mlp.tile([SPC, 128], f32, tag="h2")
                for k in range(4):
                    nc.tensor.matmul(h2ps, h1T[:, k, :], w2_sb[:, k, :],
                                     start=(k == 0), stop=False)
                nc.tensor.matmul(h2ps, onesf_sb, b2_sb, start=False, stop=True)
                h2 = sg.tile([SPC, 128], f16)
                nc.scalar.activation(out=h2, in_=h2ps, func=Act.Gelu)

                tp = tpp.tile([128, 16], f32, tag="tp")
                nc.tensor.matmul(tp[:, 0:SPC], h2, idn16_sb[0:SPC, 0:SPC],
                                 start=True, stop=True)
                h2T = sg.tile([128, SPC], f32)
                nc.vector.tensor_copy(h2T, tp[:, 0:SPC])

                # layer 3: out[8, 5] = h2 @ W3 + b3
                ops = mlp.tile([SPC, 5], f32, tag="o")
                nc.tensor.matmul(ops, h2T, w3_sb, start=True, stop=False)
                nc.tensor.matmul(ops, onesf_sb, b3_sb, start=False, stop=True)
                o_sb = sm.tile([SPC, 5], f32)
                nc.vector.tensor_copy(o_sb, ops)
                # gpsimd queue: its stream chunks (if any) are the last-needed
                # ones, so parking the tiny out DMA here never stalls the stream
                nc.gpsimd.dma_start(out=out, in_=o_sb)

            if repeat == 1 and unroll == 1:
                full_pass()
            else:
                with tc.For_i(0, repeat, 1) as _i:
                    for _u in range(unroll):
                        full_pass(_u % 2)

    nc.compile()
    return nc


def _get_nc(NT, repeat=1, unroll=1):
    key = (NT, repeat, unroll)
    if key not in _NC_CACHE:
        _NC_CACHE[key] = _build_nc(NT, repeat, unroll)
    return _NC_CACHE[key]


def _quant_sigma_delta(Xall, group_starts):
    """Quantize [NC, R, H] f32 -> e4m3 bytes with per-64-row-chain error
    feedback; carry resets at group starts so groups stay independent."""
    NC, R, _ = Xall.shape
    X3 = Xall.reshape(-1, 64, H)
    K = X3.shape[0]
    Q = np.empty((K, 64, H), np.uint8)
    carry = np.zeros((K, H), np.float32)
    resets = [[] for _ in range(64)]
    for cc in range(NC):
        for gs in group_starts[cc]:
            g = cc * R + gs
            if g % 64:
                resets[g % 64].append(g // 64)
    resets = [np.asarray(r, np.int64) for r in resets]
    for i in range(64):
        if resets[i].size:
            carry[resets[i]] = 0.0
        t = X3[:, i] + carry
        u = t.astype(np.float16).view(np.uint16)
        Q[:, i] = BYTE_LUT[u]
        carry = t - DEQ_LUT[u]
    return Q.reshape(NC, R, H)


def _prepare(hidden, attention_mask, gamma, beta, W1, b1, W2, b2, W3, b3):
    """Host-side sharding + packing. Returns (in_maps, core_samples, NT)."""
    L = attention_mask.astype(np.int64).sum(1)          # [B]
    mid = L // 2
    rows = L - 2                                        # used rows per sample

    # balance total rows across cores (greedy LPT, exactly SPC samples/core)
    order = np.argsort(-rows, kind="stable")
    core_rows = [0] * NCORES
    core_samples = [[] for _ in range(NCORES)]
    for b in order:
        cands = sorted(range(NCORES),
                       key=lambda cc: (len(core_samples[cc]) >= SPC, core_rows[cc]))
        cc = cands[0]
        core_samples[cc].append(int(b))
        core_rows[cc] += int(rows[b])

    maxrows = max(core_rows)
    NT = 2 * max(1, -(-maxrows // 256))   # even # of 128-row tiles
    R = NT * 128

    hidden2d = np.ascontiguousarray(hidden).reshape(B * S, H)
    gamma64 = np.asarray(gamma, np.float64)
    beta64 = np.asarray(beta, np.float64)
    W164 = np.asarray(W1, np.float64)
    b164 = np.asarray(b1, np.float64)
    W1a, W1b, W1c = W164[0:H], W164[H:2 * H], W164[2 * H:3 * H]
    W1f = np.concatenate([gamma64[:, None] * (W1a - W1c),
                          gamma64[:, None] * (W1b + W1c)], axis=0)
    b1f = b164 + beta64 @ (W1a + W1b)
    shared = dict(
        idn=np.eye(16, dtype=np.float32),
        w1=W1f.astype(np.float16),
        b1=b1f.astype(np.float16).reshape(1, -1),
        w2=np.ascontiguousarray(W2).astype(np.float16),
        b2=np.ascontiguousarray(b2, np.float32).reshape(1, -1),
        w3=np.ascontiguousarray(W3, np.float32),
        b3=np.ascontiguousarray(b3, np.float32).reshape(1, -1),
    )

    Xall = np.zeros((NCORES, R, H), np.float32)
    group_starts = []
    meta = []
    for cc in range(NCORES):
        samples = core_samples[cc]
        # per-group row counts: (first half, second half) per sample
        gcounts = []
        for b in samples:
            gcounts += [max(int(mid[b]) - 1, 0), max(int(L[b]) - 1 - int(mid[b]), 0)]
        starts = np.concatenate([[0], np.cumsum(gcounts)])[:-1]
        group_starts.append(starts.astype(np.int64))
        idx = np.concatenate([b * S + np.arange(1, int(L[b]) - 1) for b in samples])
        Rc = idx.size
        Xall[cc, :Rc] = hidden2d[idx]
        meta.append((samples, Rc))

    Qall = _quant_sigma_delta(Xall, group_starts)

    in_maps = []
    for cc in range(NCORES):
        samples, Rc = meta[cc]
        rcounts = [int(rows[b]) for b in samples]
        xin = np.ascontiguousarray(
            Qall[cc].reshape(NT, 128, H).transpose(1, 0, 2)
            .reshape(128, NT * H)).view(F8)

        pos = np.concatenate([np.arange(1, int(L[b]) - 1) for b in samples])
        sj = np.repeat(np.arange(SPC), rcounts)
        mids = np.repeat([int(mid[b]) for b in samples], rcounts)
        col = np.where(pos < mids, sj, sj + SPC)
        m = np.zeros((R, 16), np.uint8)
        m[np.arange(Rc), col] = _ONE_BYTE     # e4m3 bit pattern for 1.0
        mc = np.ascontiguousarray(
            m.reshape(NT, 128, 16).transpose(1, 0, 2).reshape(128, NT * 16)).view(F8)

        cnt1 = np.array([max(int(mid[b]) - 1, 1) for b in samples], np.float64)
        cnt2 = np.array([max(int(L[b]) - 1 - int(mid[b]), 1) for b in samples],
                        np.float64)
        epsc = np.concatenate([1e-5 * cnt1 ** 2, 1e-5 * cnt2 ** 2])
        epsc = epsc.astype(np.float32).reshape(16, 1)

        in_maps.append(dict(xin=xin, mc=mc, epsc=epsc, **shared))
    return in_maps, core_samples, NT


def kernel(**inputs):
    from concourse.bass_utils import run_bass_kernel_spmd

    args = {k: np.asarray(v) for k, v in inputs.items()}
    in_maps, core_samples, NT = _prepare(
        args["hidden"].astype(np.float32, copy=False),
        args["attention_mask"],
        args["gamma"], args["beta"],
        args["W1"], args["b1"], args["W2"], args["b2"], args["W3"], args["b3"],
    )
    nc = _get_nc(NT)
    res = run_bass_kernel_spmd(nc, in_maps, core_ids=list(range(NCORES)))
    out = np.zeros((B, 5), np.float32)
    for cc in range(NCORES):
        o = res.results[cc]["out"]
        for j, b in enumerate(core_samples[cc]):
            out[b] = o[j]
    return out
